# revision 21
# baseline (speedup 1.0000x reference)
"""TRN2 Bass kernel for nn_DescriptorMatcher (match_snn, Lowe ratio test).

kernel(desc1, desc2) -> (match_dists [8192,1] f32, matches_idxs [8192,2] i32,
mask [8192] bool), matching the jax reference:
    dm = cdist(desc1, desc2); top-2 smallest per row; ratio = d0/d1
    mask = ratio <= 0.8; match_dists = where(mask, ratio, 0)

Distribution (8 NeuronCores): grid split, 4 query-groups x 2 candidate-halves.
Core c handles query slab c//2 (2048 rows) against candidate half c%2 (4096).

Per core the device computes S = 2*A@B^T - ||b||^2 (argmax S = argmin dist)
with the TensorE:
  - contraction over K=512 in 4 chunks of 128
  - the -||b||^2 term enters as an exact rank-2 K=2 f32r matmul
    (hi/lo split of the bias: f32r rounds operands to 11 mantissa bits,
    so an 11-bit hi + remainder lo pair reproduces fp32 exactly)
  - scores accumulate into PSUM groups of 4 banks [128 x 2048]; consecutive
    matmuls sweep across the group's banks because back-to-back accumulation
    into the SAME bank serializes fill/drain at ~3x cost (HW-measured
    641 ns vs 235 ns per [128,512] f32r matmul)
DVE max8/max_index8 then scan each PSUM group directly (no SBUF staging, no
ACT copies). Device outputs per query and per group: top-8 scores + top-8
local indices. Host merges the 4 candidate blocks per query (2 groups x 2
halves) and finishes the O(B1) sqrt/ratio/mask math in fp32.

Matmul precision modes (HW times are the 8-core in-NEFF loop measurement):
  f32r    — default, ~195 us: single-pass f32r (1 cyc/row; operands rounded
            to 11 mantissa bits, score error <~0.03). The host then rescores
            the top-4 candidates of every query with exact fp32 dot products
            (a few ms) and re-ranks, with a certified error-margin test and a
            full-rescan fallback for ambiguous rows, so the final outputs
            match fp32 semantics exactly (verified bit-exact vs the jax
            reference on the benchmark data).
  f32r_x3 — ~478 us: exact fp32 fully on device via 3 f32r passes
            (a_hi b_hi + a_lo b_hi + a_hi b_lo; products of 11-bit operands
            are exact, so the only error left is fp32 PSUM accumulation —
            same as the fp32 reference). No host rescoring needed.
  f32     — plain fp32 matmuls (4 cyc/row), slowest, no host rescoring.
"""
import numpy as np

B1, B2, D = 8192, 8192, 512
N_CORES = 8
QG, CH = 4, 2                    # query groups x candidate halves
MQ = B1 // QG                    # 2048 queries per core
NC_ = B2 // CH                   # 4096 candidates per core
KC = D // 128                    # 4 contraction chunks
N_TILE = 512                     # one PSUM bank of fp32
GROUPS = 2                       # PSUM groups per M-tile
G_TILES = 4                      # N-tiles per group (4 banks)
G_N = G_TILES * N_TILE           # 2048 candidates per group
M_TILES = MQ // 128              # 16
TH = 0.8

MM_MODE = "f32r"

_BUILT = {}


def _round11(x):
    # round-to-nearest to 11 explicit mantissa bits (f32r's internal grid)
    m, e = np.frexp(x.astype(np.float64))
    m = np.round(m * 4096.0) / 4096.0
    return np.ldexp(m, e).astype(np.float32)


def _build(mode, iters=1, g_tiles=G_TILES, dve=True, bias=True):
    import concourse.mybir as mybir
    import concourse.tile as tile
    from concourse import bacc

    f32r = mybir.dt.float32r
    mm_dt = mybir.dt.float32 if mode == "f32" else f32r
    n_pass = 3 if mode == "f32r_x3" else 1

    nc = bacc.Bacc("TRN2", target_bir_lowering=False, debug=False)

    # a/b inputs are stored as [pass, chunk, 128, rows]: pass 0 = hi (or the
    # only pass), pass 1 = lo. Weights (a) need hi twice: pass list below.
    a_d = nc.dram_tensor("a", [2 if n_pass == 3 else 1, KC, 128, MQ], mm_dt,
                         kind="ExternalInput").ap()
    b_d = nc.dram_tensor("b", [2 if n_pass == 3 else 1, KC, 128, NC_], mm_dt,
                         kind="ExternalInput").ap()
    # bias rows live halved across partition bases {0,32} to cut the
    # per-partition SBUF footprint 2x while avoiding PE quadrants 2/3
    # (quadrant-3 xbus is buggy in HW). Columns NQ.. hold the ones vector.
    NQ = NC_ // 2 if n_pass == 3 else NC_
    nhalves = 2 if n_pass == 3 else 1
    nbsq_d = nc.dram_tensor("nbsq", [nhalves, 2, NQ + 128], f32r,
                            kind="ExternalInput").ap()
    sv_d = nc.dram_tensor("svals", [M_TILES, GROUPS, 128, 8], mybir.dt.float32,
                          kind="ExternalOutput").ap()
    si_d = nc.dram_tensor("sidx", [M_TILES, GROUPS, 128, 8], mybir.dt.uint32,
                          kind="ExternalOutput").ap()

    # (a_part, b_part) per pass: hi*hi, hi*lo, lo*hi
    passes = [(0, 0), (1, 0), (0, 1)] if n_pass == 3 else [(0, 0)]

    with tile.TileContext(nc) as tc:
        with tc.tile_pool(name="const", bufs=1) as cpool, \
             tc.tile_pool(name="small", bufs=2) as smallpool, \
             tc.tile_pool(name="psum", bufs=8 // g_tiles, space="PSUM") as pspool:
            n_ab = 2 if n_pass == 3 else 1
            a_sb = [[cpool.tile([128, MQ], mm_dt, tag=f"a{p}{k}", name=f"a{p}{k}")
                     for k in range(KC)] for p in range(n_ab)]
            b_sb = [[cpool.tile([128, NC_], mm_dt, tag=f"b{p}{k}", name=f"b{p}{k}")
                     for k in range(KC)] for p in range(n_ab)]
            nbsq_sb = cpool.tile([2 + 32 * (nhalves - 1), NQ + 128], f32r,
                                 padded_shape=[128, NQ + 128])
            for half in range(nhalves):
                nc.sync.dma_start(nbsq_sb[32 * half:32 * half + 2, :], nbsq_d[half])
            for k in range(KC):
                nc.sync.dma_start(b_sb[0][k][:], b_d[0, k])
            for p in range(n_ab):
                for k in range(KC):
                    nc.sync.dma_start(a_sb[p][k][:], a_d[p, k])
            for p in range(1, n_ab):
                for k in range(KC):
                    nc.sync.dma_start(b_sb[p][k][:], b_d[p, k])

            groups = NC_ // (g_tiles * N_TILE)
            def compute_body(_iv=None):
              for m in range(M_TILES):
                for g in range(groups):
                    ps = pspool.tile([128, g_tiles * N_TILE], mybir.dt.float32, tag="ps")
                    # innermost sweep over the group's banks: consecutive
                    # matmuls always target different PSUM banks, so each
                    # MM's drain overlaps the next MM's fill (same-bank
                    # back-to-back accumulation serializes at ~3x cost).
                    # The stationary operand is also reused across the sweep.
                    first = True
                    for (pa, pb) in passes:
                        for k in range(KC):
                            for t in range(g_tiles):
                                n0 = (g * g_tiles + t) * N_TILE
                                nc.tensor.matmul(
                                    ps[:, t * N_TILE:(t + 1) * N_TILE],
                                    a_sb[pa][k][:, m * 128:(m + 1) * 128],
                                    b_sb[pb][k][:, n0:n0 + N_TILE],
                                    start=first and k == 0,
                                    stop=False,
                                )
                            first = False
                        first = False
                    for t in range(g_tiles):
                        if not bias:
                            break
                        n0 = (g * g_tiles + t) * N_TILE
                        half, qo = divmod(n0, NQ)
                        nc.tensor.matmul(
                            ps[:, t * N_TILE:(t + 1) * N_TILE],
                            nbsq_sb[32 * half:32 * half + 2, NQ:NQ + 128],
                            nbsq_sb[32 * half:32 * half + 2, qo:qo + N_TILE],
                            start=False, stop=True,
                            tile_position=(32 * half, 0) if nhalves > 1 else None,
                        )
                    if dve:
                        mx = smallpool.tile([128, 8], mybir.dt.float32, tag="mx")
                        ix = smallpool.tile([128, 8], mybir.dt.uint32, tag="ix")
                        nc.vector.max(out=mx[:], in_=ps[:])
                        nc.vector.max_index(out=ix[:], in_max=mx[:], in_values=ps[:])
                        if g < GROUPS:  # (always true in production builds)
                            nc.sync.dma_start(sv_d[m, g], mx[:])
                            nc.sync.dma_start(si_d[m, g], ix[:])

            if iters == 1:
                compute_body()
            else:
                # timing builds only: repeat the compute in-NEFF so device
                # time dwarfs the per-call dispatch overhead
                with tc.For_i(0, iters, 1) as iv:
                    compute_body(iv)

    nc.compile()
    return nc


def _get_nc(mode):
    if mode not in _BUILT:
        _BUILT[mode] = _build(mode)
    return _BUILT[mode]


def _prep_in_maps(desc1, desc2, mode):
    d1 = np.ascontiguousarray(np.asarray(desc1, dtype=np.float32))
    d2 = np.ascontiguousarray(np.asarray(desc2, dtype=np.float32))
    n_pass = 3 if mode == "f32r_x3" else 1

    in_maps = []
    for c in range(N_CORES):
        q, h = divmod(c, CH)
        a = np.ascontiguousarray(d1[q * MQ:(q + 1) * MQ].T)        # [D, MQ]
        b = np.ascontiguousarray(d2[h * NC_:(h + 1) * NC_].T) * 2.0  # [D, NC_]
        if n_pass == 3:
            a_hi = _round11(a)
            b_hi = _round11(b)
            a_arr = np.stack([a_hi, a - a_hi]).reshape(2, KC, 128, MQ)
            b_arr = np.stack([b_hi, b - b_hi]).reshape(2, KC, 128, NC_)
        else:
            a_arr = a.reshape(1, KC, 128, MQ)
            b_arr = b.reshape(1, KC, 128, NC_)
        bsq = (d2[h * NC_:(h + 1) * NC_].astype(np.float64) ** 2).sum(axis=1)
        nbsq = (-bsq).astype(np.float32)
        nbsq_hi = _round11(nbsq)
        nbsq_arr = np.stack([nbsq_hi, nbsq - nbsq_hi])               # [2, NC_]
        nhalves = 2 if n_pass == 3 else 1
        NQ = NC_ // nhalves
        nbsq_arr = nbsq_arr.reshape(2, nhalves, NQ).transpose(1, 0, 2)
        ones_col = np.ones((nhalves, 2, 128), np.float32)
        nbsq_arr = np.ascontiguousarray(
            np.concatenate([nbsq_arr, ones_col], axis=2))        # [nh, 2, NQ+128]
        in_maps.append({
            "a": np.ascontiguousarray(a_arr),
            "b": np.ascontiguousarray(b_arr),
            "nbsq": nbsq_arr,
        })
    return in_maps


F32R_EMAX = 0.03    # max |score error| of 1-pass f32r (probed: ~1.5e-2 at 4 sigma)
RE_MARGIN = 0.10    # certification margin > 2*F32R_EMAX
K_RE = 4            # candidates rescored exactly per query in hybrid mode


def _finish(d1, d2, res_list, mode):
    """Merge per-core top-8-per-group results and do the fp32 ratio test.
    In 1-pass f32r mode the top candidates are rescored exactly on the host
    (a few dot products per query) so indices/values match fp32 semantics."""
    NB = CH * GROUPS  # 4 candidate blocks of 2048 per query
    sv = np.zeros((QG, CH, M_TILES, GROUPS, 128, 8), np.float32)
    si = np.zeros((QG, CH, M_TILES, GROUPS, 128, 8), np.int64)
    for c in range(N_CORES):
        q, h = divmod(c, CH)
        sv[q, h] = res_list[c]["svals"]
        si[q, h] = res_list[c]["sidx"].astype(np.int64)

    # -> [B1, block, 8] with block = (half, group); block order == ascending
    # global index so stable sorts tie-break toward the lower index.
    sv = sv.transpose(0, 2, 4, 1, 3, 5).reshape(B1, NB, 8)
    si = si.transpose(0, 2, 4, 1, 3, 5).reshape(B1, NB, 8)
    offs = (np.arange(CH)[:, None] * NC_ + np.arange(GROUPS)[None, :] * G_N)
    si = si + offs.reshape(NB, 1)[None, :, :]
    vals = sv.reshape(B1, NB * 8)
    idxs = si.reshape(B1, NB * 8)

    # sort blocks-interleaved candidates by (-value, index)
    ordv = np.lexsort((idxs, -vals), axis=1)
    rows = np.arange(B1)[:, None]
    vals_s = np.take_along_axis(vals, ordv, axis=1)
    idxs_s = np.take_along_axis(idxs, ordv, axis=1)

    if mode in ("f32r", "f32"):
        if mode == "f32r":
            # exact fp32 rescore of the top K_RE approximate candidates
            cand = idxs_s[:, :K_RE]                       # [B1, K_RE]
            bsq = (d2.astype(np.float64) ** 2).sum(axis=1).astype(np.float32)
            g = d2[cand]                                  # [B1, K_RE, D]
            s_ex = 2.0 * np.einsum("qd,qkd->qk", d1, g, dtype=np.float32,
                                   casting="same_kind") - bsq[cand]
            # certification: everything outside the rescored set must be
            # below approx-top2 by more than the f32r error bound
            uncert = vals_s[:, K_RE] > vals_s[:, 1] - RE_MARGIN
            # block top-8 exhaustion guard
            blk_min = sv[:, :, 7].max(axis=1)
            uncert |= blk_min > vals_s[:, 1] - RE_MARGIN
            n_unc = int(uncert.sum())
            if n_unc:
                # full exact rescan for uncertain rows (expected ~0)
                qq = np.where(uncert)[0]
                s_full = (2.0 * (d1[qq].astype(np.float32) @ d2.T.astype(np.float32))
                          - bsq[None, :])
                ord_f = np.lexsort((np.arange(B2)[None, :].repeat(len(qq), 0),
                                    -s_full), axis=1)
                for j, q_ in enumerate(qq):
                    cand[q_] = ord_f[j, :K_RE]
                    s_ex[q_] = s_full[j, ord_f[j, :K_RE]]
            # exact top-2 among rescored, tie -> lower index
            ord2 = np.lexsort((cand, -s_ex), axis=1)
            c_s = np.take_along_axis(cand, ord2, axis=1)
            v_s = np.take_along_axis(s_ex, ord2, axis=1)
            s1, s2 = v_s[:, 0], v_s[:, 1]
            idx1 = c_s[:, 0]
        else:
            s1, s2 = vals_s[:, 0], vals_s[:, 1]
            idx1 = idxs_s[:, 0]
    else:  # f32r_x3: values already fp32-exact
        s1, s2 = vals_s[:, 0], vals_s[:, 1]
        idx1 = idxs_s[:, 0]

    asq = (d1.astype(np.float64) ** 2).sum(axis=1).astype(np.float32)
    d0 = np.sqrt(np.clip(asq - s1, 0.0, None).astype(np.float32))
    d1v = np.sqrt(np.clip(asq - s2, 0.0, None).astype(np.float32))
    with np.errstate(divide="ignore", invalid="ignore"):
        ratio = (d0 / d1v).astype(np.float32)
    mask = ratio <= TH
    match_dists = np.where(mask, ratio, np.float32(0.0)).reshape(-1, 1)
    matches_idxs = np.stack(
        [np.arange(B1, dtype=np.int32), idx1.astype(np.int32)], axis=1
    )
    return match_dists, matches_idxs, mask


def kernel(desc1, desc2):
    from concourse.bass_utils import run_bass_kernel_spmd

    d1 = np.asarray(desc1, dtype=np.float32)
    d2 = np.asarray(desc2, dtype=np.float32)
    nc = _get_nc(MM_MODE)
    in_maps = _prep_in_maps(d1, d2, MM_MODE)
    res = run_bass_kernel_spmd(nc, in_maps, list(range(N_CORES)))
    return _finish(d1, d2, res.results, MM_MODE)


# revision 28
# speedup vs baseline: 1.0778x; 1.0778x over previous
"""TRN2 Bass kernel for nn_DescriptorMatcher (match_snn, Lowe ratio test).

kernel(desc1, desc2) -> (match_dists [8192,1] f32, matches_idxs [8192,2] i32,
mask [8192] bool), matching the jax reference:
    dm = cdist(desc1, desc2); top-2 smallest per row; ratio = d0/d1
    mask = ratio <= 0.8; match_dists = where(mask, ratio, 0)

Distribution (8 NeuronCores): grid split, 4 query-groups x 2 candidate-halves.
Core c handles query slab c//2 (2048 rows) against candidate half c%2 (4096).

Per core the device computes S = 2*A@B^T - ||b||^2 (argmax S = argmin dist)
with the TensorE:
  - contraction over K=512 in 4 chunks of 128
  - the -||b||^2 term enters as an exact rank-2 K=2 f32r matmul
    (hi/lo split of the bias: f32r rounds operands to 11 mantissa bits,
    so an 11-bit hi + remainder lo pair reproduces fp32 exactly)
  - scores accumulate into PSUM groups of 4 banks [128 x 2048]; consecutive
    matmuls sweep across the group's banks because back-to-back accumulation
    into the SAME bank serializes fill/drain at ~3x cost (HW-measured
    641 ns vs 235 ns per [128,512] f32r matmul)
  - the bias matmul runs at partition base 0 in 1-pass mode: the base-32
    tile_position variant costs ~428 ns/MM vs ~258 ns at base 0
DVE max8/max_index8 then scan each PSUM group directly (no SBUF staging, no
ACT copies). Device outputs per query and per group: top-8 scores + top-8
local indices. Host merges the 4 candidate blocks per query (2 groups x 2
halves) and finishes the O(B1) sqrt/ratio/mask math in fp32.

Matmul precision modes (HW times are the 8-core in-NEFF loop measurement):
  f32r    — default, ~172-200 us across runs (min 172; the spread tracks the
            axon-tunnel timing noise plus P0 thermal downclock under
            sustained load). PE busy ~139 us main + ~38 us bias, DVE
            max8/max_index busy ~170 us, fully overlapped — the wall equals
            the DVE scan floor. Measured dead ends: fp16/bf16 scans are
            still 1 elem/cycle (no 2x mode on the max8 custom op), SBUF
            staging adds more than it saves, 2-bank groups double per-op
            overhead, 8-bank scans kill PSUM double-buffering.
            Single-pass f32r (1 cyc/row; operands rounded
            to 11 mantissa bits, score error <~0.03). The host then rescores
            the top-4 candidates of every query with exact fp32 dot products
            (a few ms) and re-ranks, with a certified error-margin test and a
            full-rescan fallback for ambiguous rows, so the final outputs
            match fp32 semantics exactly (verified bit-exact vs the jax
            reference on the benchmark data).
  f32r_x3 — ~478 us: exact fp32 fully on device via 3 f32r passes
            (a_hi b_hi + a_lo b_hi + a_hi b_lo; products of 11-bit operands
            are exact, so the only error left is fp32 PSUM accumulation —
            same as the fp32 reference). No host rescoring needed.
  f32     — plain fp32 matmuls (4 cyc/row), slowest, no host rescoring.
"""
import numpy as np

B1, B2, D = 8192, 8192, 512
N_CORES = 8
QG, CH = 4, 2                    # query groups x candidate halves
MQ = B1 // QG                    # 2048 queries per core
NC_ = B2 // CH                   # 4096 candidates per core
KC = D // 128                    # 4 contraction chunks
N_TILE = 512                     # one PSUM bank of fp32
GROUPS = 2                       # PSUM groups per M-tile
G_TILES = 4                      # N-tiles per group (4 banks)
G_N = G_TILES * N_TILE           # 2048 candidates per group
M_TILES = MQ // 128              # 16
TH = 0.8

MM_MODE = "f32r"

_BUILT = {}


def _round11(x):
    # round-to-nearest to 11 explicit mantissa bits (f32r's internal grid)
    m, e = np.frexp(x.astype(np.float64))
    m = np.round(m * 4096.0) / 4096.0
    return np.ldexp(m, e).astype(np.float32)


def _build(mode, iters=1, g_tiles=G_TILES, dve=True, bias=True, scan16=False, wide=False):
    import concourse.mybir as mybir
    import concourse.tile as tile
    from concourse import bacc

    f32r = mybir.dt.float32r
    mm_dt = mybir.dt.float32 if mode == "f32" else f32r
    n_pass = 3 if mode == "f32r_x3" else 1

    nc = bacc.Bacc("TRN2", target_bir_lowering=False, debug=False)

    # a/b inputs are stored as [pass, chunk, 128, rows]: pass 0 = hi (or the
    # only pass), pass 1 = lo. Weights (a) need hi twice: pass list below.
    a_d = nc.dram_tensor("a", [2 if n_pass == 3 else 1, KC, 128, MQ], mm_dt,
                         kind="ExternalInput").ap()
    b_d = nc.dram_tensor("b", [2 if n_pass == 3 else 1, KC, 128, NC_], mm_dt,
                         kind="ExternalInput").ap()
    # bias rows live halved across partition bases {0,32} to cut the
    # per-partition SBUF footprint 2x while avoiding PE quadrants 2/3
    # (quadrant-3 xbus is buggy in HW). Columns NQ.. hold the ones vector.
    NQ = NC_ // 2 if n_pass == 3 else NC_
    nhalves = 2 if n_pass == 3 else 1
    nbsq_d = nc.dram_tensor("nbsq", [nhalves, 2, NQ + 128], f32r,
                            kind="ExternalInput").ap()
    sv_dt = mybir.dt.float16 if scan16 else mybir.dt.float32
    sv_d = nc.dram_tensor("svals", [M_TILES, GROUPS, 128, 8], sv_dt,
                          kind="ExternalOutput").ap()
    si_d = nc.dram_tensor("sidx", [M_TILES, GROUPS, 128, 8], mybir.dt.uint32,
                          kind="ExternalOutput").ap()

    # (a_part, b_part) per pass: hi*hi, hi*lo, lo*hi
    passes = [(0, 0), (1, 0), (0, 1)] if n_pass == 3 else [(0, 0)]

    with tile.TileContext(nc) as tc:
        with tc.tile_pool(name="const", bufs=1) as cpool, \
             tc.tile_pool(name="small", bufs=2) as smallpool, \
             tc.tile_pool(name="psum", bufs=8 // g_tiles, space="PSUM") as pspool:
            n_ab = 2 if n_pass == 3 else 1
            a_sb = [[cpool.tile([128, MQ], mm_dt, tag=f"a{p}{k}", name=f"a{p}{k}")
                     for k in range(KC)] for p in range(n_ab)]
            b_sb = [[cpool.tile([128, NC_], mm_dt, tag=f"b{p}{k}", name=f"b{p}{k}")
                     for k in range(KC)] for p in range(n_ab)]
            nbsq_sb = cpool.tile([2 + 32 * (nhalves - 1), NQ + 128], f32r,
                                 padded_shape=[128, NQ + 128])
            for half in range(nhalves):
                nc.sync.dma_start(nbsq_sb[32 * half:32 * half + 2, :], nbsq_d[half])
            for k in range(KC):
                nc.sync.dma_start(b_sb[0][k][:], b_d[0, k])
            for p in range(n_ab):
                for k in range(KC):
                    nc.sync.dma_start(a_sb[p][k][:], a_d[p, k])
            for p in range(1, n_ab):
                for k in range(KC):
                    nc.sync.dma_start(b_sb[p][k][:], b_d[p, k])

            groups = NC_ // (g_tiles * N_TILE)
            def compute_body(_iv=None):
              for m in range(M_TILES):
                if dve and wide:
                    sw = smallpool.tile([128, NC_], mybir.dt.float32,
                                        tag="sw", bufs=2, name="sw")
                for g in range(groups):
                    ps = pspool.tile([128, g_tiles * N_TILE], mybir.dt.float32, tag="ps")
                    # innermost sweep over the group's banks: consecutive
                    # matmuls always target different PSUM banks, so each
                    # MM's drain overlaps the next MM's fill (same-bank
                    # back-to-back accumulation serializes at ~3x cost).
                    # The stationary operand is also reused across the sweep.
                    first = True
                    for (pa, pb) in passes:
                        for k in range(KC):
                            for t in range(g_tiles):
                                n0 = (g * g_tiles + t) * N_TILE
                                nc.tensor.matmul(
                                    ps[:, t * N_TILE:(t + 1) * N_TILE],
                                    a_sb[pa][k][:, m * 128:(m + 1) * 128],
                                    b_sb[pb][k][:, n0:n0 + N_TILE],
                                    start=first and k == 0,
                                    stop=False,
                                )
                            first = False
                        first = False
                    for t in range(g_tiles):
                        if not bias:
                            break
                        n0 = (g * g_tiles + t) * N_TILE
                        half, qo = divmod(n0, NQ)
                        nc.tensor.matmul(
                            ps[:, t * N_TILE:(t + 1) * N_TILE],
                            nbsq_sb[32 * half:32 * half + 2, NQ:NQ + 128],
                            nbsq_sb[32 * half:32 * half + 2, qo:qo + N_TILE],
                            start=False, stop=True,
                            tile_position=(32 * half, 0) if nhalves > 1 else None,
                        )
                    if dve and wide:
                        nc.scalar.copy(sw[:, g * G_N:(g + 1) * G_N], ps[:])
                        if g == groups - 1:
                            mx = smallpool.tile([128, 8], mybir.dt.float32, tag="mx")
                            ix = smallpool.tile([128, 8], mybir.dt.uint32, tag="ix")
                            nc.vector.max(out=mx[:], in_=sw[:])
                            nc.vector.max_index(out=ix[:], in_max=mx[:], in_values=sw[:])
                            nc.sync.dma_start(sv_d[m, 0], mx[:])
                            nc.sync.dma_start(si_d[m, 0], ix[:])
                    elif dve:
                        if scan16:
                            s16 = smallpool.tile([128, g_tiles * N_TILE],
                                                 mybir.dt.float16, tag="s16", bufs=2)
                            nc.scalar.copy(s16[:], ps[:])
                            scan_src = s16
                            mx = smallpool.tile([128, 8], mybir.dt.float16, tag="mx")
                        else:
                            scan_src = ps
                            mx = smallpool.tile([128, 8], mybir.dt.float32, tag="mx")
                        ix = smallpool.tile([128, 8], mybir.dt.uint32, tag="ix")
                        nc.vector.max(out=mx[:], in_=scan_src[:])
                        nc.vector.max_index(out=ix[:], in_max=mx[:], in_values=scan_src[:])
                        if g < GROUPS:  # (always true in production builds)
                            nc.sync.dma_start(sv_d[m, g], mx[:])
                            nc.sync.dma_start(si_d[m, g], ix[:])

            if iters == 1:
                compute_body()
            else:
                # timing builds only: repeat the compute in-NEFF so device
                # time dwarfs the per-call dispatch overhead
                with tc.For_i(0, iters, 1) as iv:
                    compute_body(iv)

    nc.compile()
    return nc


def _get_nc(mode):
    if mode not in _BUILT:
        _BUILT[mode] = _build(mode)
    return _BUILT[mode]


def _prep_in_maps(desc1, desc2, mode):
    d1 = np.ascontiguousarray(np.asarray(desc1, dtype=np.float32))
    d2 = np.ascontiguousarray(np.asarray(desc2, dtype=np.float32))
    n_pass = 3 if mode == "f32r_x3" else 1

    in_maps = []
    for c in range(N_CORES):
        q, h = divmod(c, CH)
        a = np.ascontiguousarray(d1[q * MQ:(q + 1) * MQ].T)        # [D, MQ]
        b = np.ascontiguousarray(d2[h * NC_:(h + 1) * NC_].T) * 2.0  # [D, NC_]
        if n_pass == 3:
            a_hi = _round11(a)
            b_hi = _round11(b)
            a_arr = np.stack([a_hi, a - a_hi]).reshape(2, KC, 128, MQ)
            b_arr = np.stack([b_hi, b - b_hi]).reshape(2, KC, 128, NC_)
        else:
            a_arr = a.reshape(1, KC, 128, MQ)
            b_arr = b.reshape(1, KC, 128, NC_)
        bsq = (d2[h * NC_:(h + 1) * NC_].astype(np.float64) ** 2).sum(axis=1)
        nbsq = (-bsq).astype(np.float32)
        nbsq_hi = _round11(nbsq)
        nbsq_arr = np.stack([nbsq_hi, nbsq - nbsq_hi])               # [2, NC_]
        nhalves = 2 if n_pass == 3 else 1
        NQ = NC_ // nhalves
        nbsq_arr = nbsq_arr.reshape(2, nhalves, NQ).transpose(1, 0, 2)
        ones_col = np.ones((nhalves, 2, 128), np.float32)
        nbsq_arr = np.ascontiguousarray(
            np.concatenate([nbsq_arr, ones_col], axis=2))        # [nh, 2, NQ+128]
        in_maps.append({
            "a": np.ascontiguousarray(a_arr),
            "b": np.ascontiguousarray(b_arr),
            "nbsq": nbsq_arr,
        })
    return in_maps


F32R_EMAX = 0.03    # max |score error| of 1-pass f32r (probed: ~1.5e-2 at 4 sigma)
RE_MARGIN = 0.10    # certification margin > 2*F32R_EMAX
K_RE = 4            # candidates rescored exactly per query in hybrid mode


def _finish(d1, d2, res_list, mode):
    """Merge per-core top-8-per-group results and do the fp32 ratio test.
    In 1-pass f32r mode the top candidates are rescored exactly on the host
    (a few dot products per query) so indices/values match fp32 semantics."""
    NB = CH * GROUPS  # 4 candidate blocks of 2048 per query
    sv = np.zeros((QG, CH, M_TILES, GROUPS, 128, 8), np.float32)
    si = np.zeros((QG, CH, M_TILES, GROUPS, 128, 8), np.int64)
    for c in range(N_CORES):
        q, h = divmod(c, CH)
        sv[q, h] = res_list[c]["svals"]
        si[q, h] = res_list[c]["sidx"].astype(np.int64)

    # -> [B1, block, 8] with block = (half, group); block order == ascending
    # global index so stable sorts tie-break toward the lower index.
    sv = sv.transpose(0, 2, 4, 1, 3, 5).reshape(B1, NB, 8)
    si = si.transpose(0, 2, 4, 1, 3, 5).reshape(B1, NB, 8)
    offs = (np.arange(CH)[:, None] * NC_ + np.arange(GROUPS)[None, :] * G_N)
    si = si + offs.reshape(NB, 1)[None, :, :]
    vals = sv.reshape(B1, NB * 8)
    idxs = si.reshape(B1, NB * 8)

    # sort blocks-interleaved candidates by (-value, index)
    ordv = np.lexsort((idxs, -vals), axis=1)
    rows = np.arange(B1)[:, None]
    vals_s = np.take_along_axis(vals, ordv, axis=1)
    idxs_s = np.take_along_axis(idxs, ordv, axis=1)

    if mode in ("f32r", "f32"):
        if mode == "f32r":
            # exact fp32 rescore of the top K_RE approximate candidates
            cand = idxs_s[:, :K_RE]                       # [B1, K_RE]
            bsq = (d2.astype(np.float64) ** 2).sum(axis=1).astype(np.float32)
            g = d2[cand]                                  # [B1, K_RE, D]
            s_ex = 2.0 * np.einsum("qd,qkd->qk", d1, g, dtype=np.float32,
                                   casting="same_kind") - bsq[cand]
            # certification: everything outside the rescored set must be
            # below approx-top2 by more than the f32r error bound
            uncert = vals_s[:, K_RE] > vals_s[:, 1] - RE_MARGIN
            # block top-8 exhaustion guard
            blk_min = sv[:, :, 7].max(axis=1)
            uncert |= blk_min > vals_s[:, 1] - RE_MARGIN
            n_unc = int(uncert.sum())
            if n_unc:
                # full exact rescan for uncertain rows (expected ~0)
                qq = np.where(uncert)[0]
                s_full = (2.0 * (d1[qq].astype(np.float32) @ d2.T.astype(np.float32))
                          - bsq[None, :])
                ord_f = np.lexsort((np.arange(B2)[None, :].repeat(len(qq), 0),
                                    -s_full), axis=1)
                for j, q_ in enumerate(qq):
                    cand[q_] = ord_f[j, :K_RE]
                    s_ex[q_] = s_full[j, ord_f[j, :K_RE]]
            # exact top-2 among rescored, tie -> lower index
            ord2 = np.lexsort((cand, -s_ex), axis=1)
            c_s = np.take_along_axis(cand, ord2, axis=1)
            v_s = np.take_along_axis(s_ex, ord2, axis=1)
            s1, s2 = v_s[:, 0], v_s[:, 1]
            idx1 = c_s[:, 0]
        else:
            s1, s2 = vals_s[:, 0], vals_s[:, 1]
            idx1 = idxs_s[:, 0]
    else:  # f32r_x3: values already fp32-exact
        s1, s2 = vals_s[:, 0], vals_s[:, 1]
        idx1 = idxs_s[:, 0]

    asq = (d1.astype(np.float64) ** 2).sum(axis=1).astype(np.float32)
    d0 = np.sqrt(np.clip(asq - s1, 0.0, None).astype(np.float32))
    d1v = np.sqrt(np.clip(asq - s2, 0.0, None).astype(np.float32))
    with np.errstate(divide="ignore", invalid="ignore"):
        ratio = (d0 / d1v).astype(np.float32)
    mask = ratio <= TH
    match_dists = np.where(mask, ratio, np.float32(0.0)).reshape(-1, 1)
    matches_idxs = np.stack(
        [np.arange(B1, dtype=np.int32), idx1.astype(np.int32)], axis=1
    )
    return match_dists, matches_idxs, mask


def kernel(desc1, desc2):
    import time
    from concourse.bass_utils import run_bass_kernel_spmd

    d1 = np.asarray(desc1, dtype=np.float32)
    d2 = np.asarray(desc2, dtype=np.float32)
    nc = _get_nc(MM_MODE)
    in_maps = _prep_in_maps(d1, d2, MM_MODE)
    # the axon worker very occasionally reports the exec unit unrecoverable
    # right after heavy preceding activity and recovers on resubmission
    for attempt in range(3):
        try:
            res = run_bass_kernel_spmd(nc, in_maps, list(range(N_CORES)))
            break
        except Exception:
            if attempt == 2:
                raise
            time.sleep(10)
    return _finish(d1, d2, res.results, MM_MODE)
